# revision 42
# baseline (speedup 1.0000x reference)
"""Trainium2 Bass kernel for nn_BaseSearchBasedModel (sparse attention).

Math restructuring (exact up to rounding):
  topk   = user_seq_emb[b, indices[b,k]]                      (SWDGE gather)
  scores = topk . (A[h]^T tgt + c[h]) / 8  + const(b,h)       A = WQ WK^T, c = WK bQ
    The const(b,h) term (q.bK) is constant over the softmax axis -> drops out.
  heads  = softmax(scores) @ topk @ WV[h]                     (WV folded after softmax)
  mhta   = sum_h ctx[h] @ G[h] + bias0                        G = WV[h] WO_h
  logit  = MLP(concat(mhta, tgt))

v3: fp16 data path (gathered data, qk, exp, ctx, G) halves PE matmul
passes; per-pair PE transposes replaced by one XBAR DMA transpose per
chunk; softmax z via ones-row matmul + reciprocal + broadcast matmul;
per-chunk gathers rotate SWDGE queues 0-3 so descriptor generation of
chunk c+1 overlaps the SDMA drain of chunk c; output-head G matmuls
batched over all chunks with strided rhs.

Sharding: pure data parallel, batch 2048 -> 8 cores x 256.
"""

import sys

if "/opt/trn_rl_repo" not in sys.path:
    sys.path.insert(0, "/opt/trn_rl_repo")

import numpy as np

import concourse.bass as bass
import concourse.tile as tile
import concourse.mybir as mybir
from concourse import bacc
from concourse.bass_utils import run_bass_kernel_spmd
from concourse.masks import make_identity

F32 = mybir.dt.float32
F16 = mybir.dt.float16
I32 = mybir.dt.int32
I16 = mybir.dt.int16
AF = mybir.ActivationFunctionType

B, L, K, D, H = 2048, 1024, 128, 64, 4
N_CORES = 8
B_LOC = B // N_CORES  # 256

SINGLE_PACKET = False
QUEUE_ROT = True


def build(b_loc=B_LOC):
    """Build the per-core Bass module. b_loc must be a multiple of 32."""
    assert b_loc % 32 == 0
    n_grp = b_loc // 32            # gather idx groups of 32 samples
    n_pair = b_loc // 2            # sample pairs
    n_bt = (b_loc + 127) // 128    # 128-row tiles over the local batch
    n_chunk = b_loc // 16          # 16-sample chunks (half a gather idx group)

    nc = bacc.Bacc("TRN2", target_bir_lowering=False, debug=False, num_devices=N_CORES,
                   num_swdge_queues=4 if QUEUE_ROT else 1,
                   dynamic_dma_scratch_size=65536)

    useq = nc.dram_tensor("useq", [b_loc * L, D], F32, kind="ExternalInput").ap()
    tgt = nc.dram_tensor("tgt", [b_loc, D], F32, kind="ExternalInput").ap()
    # idx16: host-wrapped gather indices in the SWDGE addressing scheme,
    # [128, b_loc*K/16] int16, replicated into all 8 16-partition blocks.
    # idx16[16r+p, (g, s, cj)] = idx[32g+s, 16cj+p] + 1024*s  (row index into
    # the group's [32*1024, 64] slice of useq)
    idx16 = nc.dram_tensor("idx16", [128, b_loc * K // 16], I16,
                           kind="ExternalInput").ap()
    wq = nc.dram_tensor("wq", [H * D, D], F32, kind="ExternalInput").ap()
    wk = nc.dram_tensor("wk", [H * D, D], F32, kind="ExternalInput").ap()
    wv = nc.dram_tensor("wv", [H * D, D], F32, kind="ExternalInput").ap()
    bq = nc.dram_tensor("bq", [H, D], F32, kind="ExternalInput").ap()
    bv = nc.dram_tensor("bv", [H, D], F32, kind="ExternalInput").ap()
    wo = nc.dram_tensor("wo", [H * D, D], F32, kind="ExternalInput").ap()
    bo = nc.dram_tensor("bo", [D, 1], F32, kind="ExternalInput").ap()
    w1 = nc.dram_tensor("w1", [2 * D, D], F32, kind="ExternalInput").ap()
    b1 = nc.dram_tensor("b1", [D, 1], F32, kind="ExternalInput").ap()
    w2 = nc.dram_tensor("w2", [D, 1], F32, kind="ExternalInput").ap()
    b2 = nc.dram_tensor("b2", [1, 1], F32, kind="ExternalInput").ap()
    logit = nc.dram_tensor("logit", [b_loc, 1], F32, kind="ExternalOutput").ap()

    with tile.TileContext(nc) as tc, \
         tc.tile_pool(name="const", bufs=1) as const, \
         tc.tile_pool(name="stage", bufs=3) as stage, \
         tc.tile_pool(name="gath", bufs=16) as gath_pool, \
         tc.tile_pool(name="gthb", bufs=4) as gthb_pool, \
         tc.tile_pool(name="ptsb", bufs=4) as pt_pool, \
         tc.tile_pool(name="small", bufs=3) as small, \
         tc.tile_pool(name="expsb", bufs=4) as exp_pool, \
         tc.tile_pool(name="rzbsb", bufs=4) as rzb_pool, \
         tc.tile_pool(name="ps_a", bufs=1, space="PSUM") as ps_a, \
         tc.tile_pool(name="scx", bufs=3, space="PSUM") as scx_pool, \
         tc.tile_pool(name="zc", bufs=2, space="PSUM") as zc_pool, \
         tc.tile_pool(name="mhps", bufs=1, space="PSUM") as mh_pool:

        ident = const.tile([128, 128], F32, tag="ident")
        make_identity(nc, ident[:])
        ones16 = const.tile([128, 1], F16, tag="ones16")
        nc.vector.memset(ones16[:], 1.0)
        ones_row = const.tile([1, 128], F32, tag="ones_row")
        nc.vector.memset(ones_row[:], 1.0)

        # ---- index load for dma_gather ----
        # Gather order i = 32*128*g + 128*s + j; the SWDGE ucode reads idx i at
        # idxs[i % 16, i // 16] (int16); queue q's core pair (cpus 2q, 2q+1)
        # reads partitions 32q..32q+32. Host ships the indices pre-wrapped,
        # offset to the group's useq slice, and replicated — one fast DMA here.
        w16_all = const.tile([128, n_grp * 256], I16, tag="w16_all")
        nc.sync.dma_start(out=w16_all[:], in_=idx16[:])
        w16s = [w16_all[:, 256 * g:256 * (g + 1)] for g in range(n_grp)]

        # ---- weight transposes: wqT/wkT/wvT [64, 256] = [e, (h, d)] ----
        wT = {}
        for name, dram in (("wq", wq), ("wk", wk), ("wv", wv)):
            t_sb = const.tile([D, H * D], F32, tag=f"{name}T")
            for t in range(2):
                s = stage.tile([128, D], F32, tag="wstage")
                nc.sync.dma_start(out=s[:], in_=dram[128 * t:128 * (t + 1), :])
                tr = ps_a.tile([D, 128], F32, tag="pa")
                nc.tensor.transpose(tr[:], s[:], ident[:])
                nc.scalar.activation(t_sb[:, 128 * t:128 * (t + 1)], tr[:], AF.Copy)
            wT[name] = t_sb

        # ---- wo_r [64, 256] = [e, (h, f)] (reshaped, not transposed) ----
        wo_r = const.tile([D, H * D], F32, tag="wo_r")
        nc.sync.dma_start(out=wo_r[:].rearrange("p (h f) -> p h f", h=H),
                          in_=wo[:].rearrange("(h e) f -> e h f", h=H))

        # ---- small bias columns ----
        bqcol = const.tile([D, H], F32, tag="bqcol")
        nc.sync.dma_start(out=bqcol[:], in_=bq[:].rearrange("h e -> e h"))
        bvcol = const.tile([D, H], F32, tag="bvcol")
        nc.sync.dma_start(out=bvcol[:], in_=bv[:].rearrange("h e -> e h"))
        bocol = const.tile([D, 1], F32, tag="bocol")
        nc.sync.dma_start(out=bocol[:], in_=bo[:])
        b1col = const.tile([D, 1], F32, tag="b1col")
        nc.sync.dma_start(out=b1col[:], in_=b1[:])
        w1_sb = const.tile([2 * D, D], F32, tag="w1")
        nc.sync.dma_start(out=w1_sb[:], in_=w1[:])
        w2_sb = const.tile([D, 1], F32, tag="w2")
        nc.sync.dma_start(out=w2_sb[:], in_=w2[:])
        b2_sb = const.tile([1, 1], F32, tag="b2")
        nc.sync.dma_start(out=b2_sb[:], in_=b2[:])

        # ---- target transpose: tgtT [64, b_loc]; also xT rows 64:128 ----
        tgtT = const.tile([D, b_loc], F32, tag="tgtT")
        xT = const.tile([128, b_loc], F32, tag="xT")
        for t in range(n_bt):
            r0, r1 = 128 * t, min(128 * (t + 1), b_loc)
            n = r1 - r0
            s = stage.tile([128, D], F32, tag="tstage")
            nc.sync.dma_start(out=s[:n, :], in_=tgt[r0:r1, :])
            tr = ps_a.tile([D, 128], F32, tag="pa")
            nc.tensor.transpose(tr[:, :n], s[:n, :], ident[:n, :n])
            nc.scalar.activation(tgtT[:, r0:r1], tr[:, :n], AF.Copy)
            nc.scalar.activation(xT[D:2 * D, r0:r1], tr[:, :n], AF.Copy)

        # ---- per-head folded matrices ----
        # A_sb[:, 64h:64h+64] = (WQ[h] @ WK[h]^T) / 8 ; c_col = (WK[h] @ bQ[h]) / 8
        A_sb = const.tile([D, H * D], F32, tag="A")
        c_col = const.tile([128, H], F32, tag="c_col")
        c_ps = ps_a.tile([128, H], F32, tag="pa")
        for h in range(H):
            a_ps = ps_a.tile([D, D], F32, tag="pa")
            nc.tensor.matmul(a_ps[:], lhsT=wT["wq"][:, D * h:D * (h + 1)],
                             rhs=wT["wk"][:, D * h:D * (h + 1)], start=True, stop=True)
            nc.scalar.activation(A_sb[:, D * h:D * (h + 1)], a_ps[:], AF.Copy,
                                 scale=0.125)
            nc.tensor.matmul(c_ps[0:D, h:h + 1], lhsT=wT["wk"][:, D * h:D * (h + 1)],
                             rhs=bqcol[:, h:h + 1], start=True, stop=True)
            nc.tensor.matmul(c_ps[D:2 * D, h:h + 1], lhsT=wT["wk"][:, D * h:D * (h + 1)],
                             rhs=bqcol[:, h:h + 1], start=True, stop=True)
        nc.scalar.activation(c_col[:], c_ps[:], AF.Copy, scale=0.125)

        # ---- qk block-diagonal tile [128, 8*n_pair] fp16 ----
        # pair q columns 8q..8q+7: cols 0-3 = even sample heads (rows 0:64),
        # cols 4-7 = odd sample heads (rows 64:128); rest zero.
        qk_bd = const.tile([128, 8 * n_pair], F16, tag="qk_bd")
        nc.vector.memset(qk_bd[:], 0.0)
        tgtT_v = tgtT[:].rearrange("p (s two) -> p s two", two=2)
        qk_v = qk_bd[:].rearrange("p (q c) -> p q c", c=8)
        for h in range(H):
            qk_ps = ps_a.tile([128, n_pair], F32, tag="pa")
            nc.tensor.matmul(qk_ps[0:D, :], lhsT=A_sb[:, D * h:D * (h + 1)],
                             rhs=tgtT_v[:, :, 0], start=True, stop=True)
            nc.tensor.matmul(qk_ps[D:2 * D, :], lhsT=A_sb[:, D * h:D * (h + 1)],
                             rhs=tgtT_v[:, :, 1], start=True, stop=True)
            nc.scalar.activation(qk_v[0:D, :, h], qk_ps[0:D, :], AF.Identity,
                                 bias=c_col[0:D, h:h + 1], scale=1.0)
            nc.scalar.activation(qk_v[D:2 * D, :, 4 + h], qk_ps[D:2 * D, :], AF.Identity,
                                 bias=c_col[D:2 * D, h:h + 1], scale=1.0)

        # ---- G_sb [128, 256] fp16: rows 0:64 / 64:128 both hold G[h] = WV[h] @ WO_h ----
        G_sb = const.tile([128, H * D], F16, tag="G")
        for h in range(H):
            g_ps = ps_a.tile([128, D], F32, tag="pa")
            nc.tensor.matmul(g_ps[0:D, :], lhsT=wT["wv"][:, D * h:D * (h + 1)],
                             rhs=wo_r[:, D * h:D * (h + 1)], start=True, stop=True)
            nc.tensor.matmul(g_ps[D:2 * D, :], lhsT=wT["wv"][:, D * h:D * (h + 1)],
                             rhs=wo_r[:, D * h:D * (h + 1)], start=True, stop=True)
            nc.scalar.activation(G_sb[:, D * h:D * (h + 1)], g_ps[:], AF.Copy)

        # ---- bias0 = sum_h bV[h] @ WO_h + bO, as a [64, 1] column ----
        b0_ps = ps_a.tile([1, D], F32, tag="pa")
        for h in range(H):
            nc.tensor.matmul(b0_ps[:], lhsT=bvcol[:, h:h + 1],
                             rhs=wo_r[:, D * h:D * (h + 1)],
                             start=(h == 0), stop=(h == H - 1))
        b0row = stage.tile([1, D], F32, tag="b0row")
        nc.scalar.activation(b0row[:], b0_ps[:], AF.Copy)
        b0c_ps = ps_a.tile([D, 1], F32, tag="pa")
        nc.tensor.transpose(b0c_ps[:], b0row[:], ident[0:1, 0:1])
        bias0 = const.tile([D, 1], F32, tag="bias0")
        nc.vector.tensor_tensor(out=bias0[:], in0=b0c_ps[:], in1=bocol[:],
                                op=mybir.AluOpType.add)

        # ---- main loop over chunks of 16 samples ----
        ctxn = const.tile([128, 64 * n_chunk], F16, tag="ctxn")
        # ctxn col layout: (chunk c, pair q, col k) -> 64*c + 8*q + k
        ctxn_v = ctxn[:].rearrange("p (c q k) -> p c q k", c=n_chunk, k=8)
        LAG = 2

        def issue_gather(c):
            g = c // 2
            gth = gath_pool.tile([128, 16 * D], F32, tag="gath")
            for hh in range(8):
                k16 = 8 * (c % 2) + hh
                qn = ((8 * c + hh) % 4) if QUEUE_ROT else 0
                nc.gpsimd.dma_gather(
                    out_ap=gth[:, 128 * hh:128 * (hh + 1)].rearrange(
                        "p (s d) -> p s d", d=D),
                    in_ap=useq[32 * L * g:32 * L * (g + 1), :],
                    idxs_ap=w16s[g][:, 16 * k16:16 * (k16 + 1)],
                    num_idxs=2 * K, num_idxs_reg=2 * K, elem_size=D,
                    single_packet=SINGLE_PACKET, queue_num=qn)
            return gth

        def issue_front(c, gth):
            """fp16 cast + XBAR transpose for chunk c (right after its gather)."""
            gthb = gthb_pool.tile([128, 16 * D], F16, tag="gthb")
            if c % 2:
                nc.scalar.activation(gthb[:], gth[:], AF.Copy)
            else:
                nc.vector.tensor_copy(out=gthb[:], in_=gth[:])
            pt = pt_pool.tile([128, 8 * 128], F16, tag="pt")
            eng = nc.scalar if (c % 2) else nc.sync
            eng.dma_start_transpose(
                out=pt[:].rearrange("p (q j) -> p q j", j=128), in_=gthb[:])
            return gthb, pt

        def issue_compute(c, gthb, pt):
            # scores: per pair lhsT = pT pair [128(s2,d), j], rhs = qk pair cols
            sc_ps = scx_pool.tile([128, 64], F32, tag="scx")
            for q in range(8):
                Q = 8 * c + q
                nc.tensor.matmul(sc_ps[:, 8 * q:8 * (q + 1)],
                                 lhsT=pt[:, 128 * q:128 * (q + 1)],
                                 rhs=qk_bd[:, 8 * Q:8 * (Q + 1)], start=True, stop=True)
            exp_sb = exp_pool.tile([128, 64], F16, tag="exp")
            nc.scalar.activation(exp_sb[:], sc_ps[:], AF.Exp)
            # z row = ones^T @ exp ; rz = 1/z ; rzb = broadcast of rz
            z_ps = zc_pool.tile([1, 64], F32, tag="zc")
            nc.tensor.matmul(z_ps[:], lhsT=ones16[:], rhs=exp_sb[:],
                             start=True, stop=True)
            rz = small.tile([1, 64], F32, tag="rz")
            nc.vector.reciprocal(rz[:], z_ps[:])
            rzb_ps = zc_pool.tile([128, 64], F32, tag="zc")
            nc.tensor.matmul(rzb_ps[:], lhsT=ones_row[:], rhs=rz[:],
                             start=True, stop=True)
            rzb = rzb_pool.tile([128, 64], F32, tag="rzb")
            nc.scalar.activation(rzb[:], rzb_ps[:], AF.Copy)
            # ctx on UNNORMALIZED exp; 1/Z folded in the fp16 convert below.
            ctx_ps = scx_pool.tile([128, 64], F32, tag="scx")
            for q in range(8):
                nc.tensor.matmul(ctx_ps[:, 8 * q:8 * (q + 1)],
                                 lhsT=gthb[:, 128 * q:128 * (q + 1)],
                                 rhs=exp_sb[:, 8 * q:8 * (q + 1)], start=True, stop=True)
            nc.vector.tensor_tensor(out=ctxn[:, 64 * c:64 * (c + 1)], in0=ctx_ps[:],
                                    in1=rzb[:], op=mybir.AluOpType.mult)

        # software pipeline: all gathers issued up front (dedicated buffers, no
        # tile deps between them — they stream on the 4 SWDGE queues), then
        # cast+transpose per chunk with compute LAG chunks behind. The output
        # head mh accumulation runs in two halves so the first half overlaps
        # the second half's gathers.
        mh_e = mh_pool.tile([D, n_pair], F32, tag="mh_e")
        mh_o = mh_pool.tile([D, n_pair], F32, tag="mh_o")
        hc = n_chunk // 2

        def issue_mh(half):
            c0, c1 = half * hc, (half + 1) * hc
            p0, p1 = 8 * half * hc, 8 * (half + 1) * hc
            for h in range(H):
                nc.tensor.matmul(mh_e[:, p0:p1], lhsT=G_sb[0:D, D * h:D * (h + 1)],
                                 rhs=ctxn_v[0:D, c0:c1, :, h],
                                 start=(h == 0), stop=(h == H - 1))
            for h in range(H):
                nc.tensor.matmul(mh_o[:, p0:p1], lhsT=G_sb[D:2 * D, D * h:D * (h + 1)],
                                 rhs=ctxn_v[D:2 * D, c0:c1, :, 4 + h],
                                 start=(h == 0), stop=(h == H - 1))

        gths = {c: issue_gather(c) for c in range(n_chunk)}
        fronts = {}
        for c in range(n_chunk + LAG):
            if c < n_chunk:
                fronts[c] = issue_front(c, gths[c])
            if c >= LAG:
                cc = c - LAG
                issue_compute(cc, *fronts.pop(cc))
                if cc == hc - 1:
                    issue_mh(0)
        issue_mh(1)

        # ---- output head tail: bias + MLP (fp32) ----
        x_v = xT[:].rearrange("p (s two) -> p s two", two=2)
        nc.scalar.activation(x_v[0:D, :, 0], mh_e[:], AF.Identity, bias=bias0[:],
                             scale=1.0)
        nc.scalar.activation(x_v[0:D, :, 1], mh_o[:], AF.Identity, bias=bias0[:],
                             scale=1.0)
        h1_ps = ps_a.tile([D, b_loc], F32, tag="pa")
        nc.tensor.matmul(h1_ps[:], lhsT=w1_sb[:], rhs=xT[:], start=True, stop=True)
        h1_sb = const.tile([D, b_loc], F32, tag="h1")
        nc.scalar.activation(h1_sb[:], h1_ps[:], AF.Relu, bias=b1col[:], scale=1.0)
        lg_ps = ps_a.tile([1, b_loc], F32, tag="pa")
        nc.tensor.matmul(lg_ps[:], lhsT=w2_sb[:], rhs=h1_sb[:], start=True, stop=True)
        lg_sb = const.tile([1, b_loc], F32, tag="lg")
        nc.scalar.activation(lg_sb[:], lg_ps[:], AF.Identity, bias=b2_sb[:], scale=1.0)
        nc.sync.dma_start(out=logit[:], in_=lg_sb[:])

    nc.compile()
    return nc


def make_in_maps(inputs, b_loc=B_LOC, n_cores=N_CORES):
    """Shard full inputs into per-core in_maps (data parallel over batch)."""
    idx = np.asarray(inputs["indices"]).astype(np.int32)
    # SWDGE wrapped layout: idx16[p, (g, s, cj)] = idx[32g+s, 16cj+p] + 1024 s
    # (row index into the group's [32*1024, 64] useq slice), int16, replicated
    # across the 8 16-partition blocks.
    n_grp = b_loc // 32
    idxg = idx.reshape(n_cores * n_grp, 32, K // 16, 16) + \
        (L * np.arange(32, dtype=np.int32))[None, :, None, None]
    idx16_all = np.ascontiguousarray(
        idxg.transpose(3, 0, 1, 2)).astype(np.int16)  # [16, G, 32, 8]
    idx16_all = np.broadcast_to(
        idx16_all[None], (8,) + idx16_all.shape)  # replicate to 128 partitions
    useq = np.ascontiguousarray(np.asarray(inputs["user_seq_emb"], dtype=np.float32))
    tgt = np.ascontiguousarray(np.asarray(inputs["target_emb"], dtype=np.float32)[:, 0, :])
    shared = {
        "wq": np.ascontiguousarray(np.asarray(inputs["WQ"], np.float32).reshape(H * D, D)),
        "wk": np.ascontiguousarray(np.asarray(inputs["WK"], np.float32).reshape(H * D, D)),
        "wv": np.ascontiguousarray(np.asarray(inputs["WV"], np.float32).reshape(H * D, D)),
        "bq": np.ascontiguousarray(np.asarray(inputs["bQ"], np.float32)),
        "bv": np.ascontiguousarray(np.asarray(inputs["bV"], np.float32)),
        "wo": np.ascontiguousarray(np.asarray(inputs["WO"], np.float32)),
        "bo": np.asarray(inputs["bO"], np.float32).reshape(D, 1).copy(),
        "w1": np.ascontiguousarray(np.asarray(inputs["W1"], np.float32)),
        "b1": np.asarray(inputs["b1"], np.float32).reshape(D, 1).copy(),
        "w2": np.ascontiguousarray(np.asarray(inputs["W2"], np.float32)),
        "b2": np.asarray(inputs["b2"], np.float32).reshape(1, 1).copy(),
    }
    in_maps = []
    for c in range(n_cores):
        s = slice(c * b_loc, (c + 1) * b_loc)
        m = dict(shared)
        m["useq"] = useq[s].reshape(b_loc * L, D)
        m["tgt"] = tgt[s]
        m["idx16"] = np.ascontiguousarray(
            idx16_all[:, :, c * n_grp:(c + 1) * n_grp]).reshape(128, b_loc * K // 16)
        in_maps.append(m)
    return in_maps


_NC_CACHE = {}


def kernel(**inputs):
    if B_LOC not in _NC_CACHE:
        _NC_CACHE[B_LOC] = build(B_LOC)
    nc = _NC_CACHE[B_LOC]
    in_maps = make_in_maps(inputs)
    res = run_bass_kernel_spmd(nc, in_maps, core_ids=list(range(N_CORES)))
    return np.concatenate([res.results[c]["logit"] for c in range(N_CORES)], axis=0)


# revision 43
# speedup vs baseline: 1.2386x; 1.2386x over previous
"""Trainium2 Bass kernel for nn_BaseSearchBasedModel (sparse attention).

Math restructuring (exact up to rounding):
  topk   = user_seq_emb[b, indices[b,k]]                      (SWDGE gather)
  scores = topk . (A[h]^T tgt + c[h]) / 8  + const(b,h)       A = WQ WK^T, c = WK bQ
    The const(b,h) term (q.bK) is constant over the softmax axis -> drops out.
  heads  = softmax(scores) @ topk @ WV[h]                     (WV folded after softmax)
  mhta   = sum_h ctx[h] @ G[h] + bias0                        G = WV[h] WO_h
  logit  = MLP(concat(mhta, tgt))

v3: fp16 data path (gathered data, qk, exp, ctx, G) halves PE matmul
passes; per-pair PE transposes replaced by one XBAR DMA transpose per
chunk; softmax z via ones-row matmul + reciprocal + broadcast matmul;
per-chunk gathers rotate SWDGE queues 0-3 so descriptor generation of
chunk c+1 overlaps the SDMA drain of chunk c; output-head G matmuls
batched over all chunks with strided rhs.

Sharding: pure data parallel, batch 2048 -> 8 cores x 256.
"""

import sys

if "/opt/trn_rl_repo" not in sys.path:
    sys.path.insert(0, "/opt/trn_rl_repo")

import numpy as np

import concourse.bass as bass
import concourse.tile as tile
import concourse.mybir as mybir
from concourse import bacc
from concourse.bass_utils import run_bass_kernel_spmd
from concourse.masks import make_identity

F32 = mybir.dt.float32
F16 = mybir.dt.float16
I32 = mybir.dt.int32
I16 = mybir.dt.int16
AF = mybir.ActivationFunctionType

B, L, K, D, H = 2048, 1024, 128, 64, 4
N_CORES = 8
B_LOC = B // N_CORES  # 256

SINGLE_PACKET = False
QUEUE_ROT = True


def build(b_loc=B_LOC):
    """Build the per-core Bass module. b_loc must be a multiple of 32."""
    assert b_loc % 32 == 0
    n_grp = b_loc // 32            # gather idx groups of 32 samples
    n_pair = b_loc // 2            # sample pairs
    n_bt = (b_loc + 127) // 128    # 128-row tiles over the local batch
    n_chunk = b_loc // 16          # 16-sample chunks (half a gather idx group)

    nc = bacc.Bacc("TRN2", target_bir_lowering=False, debug=False, num_devices=N_CORES,
                   num_swdge_queues=4 if QUEUE_ROT else 1,
                   dynamic_dma_scratch_size=65536)

    useq = nc.dram_tensor("useq", [b_loc * L, D], F32, kind="ExternalInput").ap()
    tgt = nc.dram_tensor("tgt", [b_loc, D], F32, kind="ExternalInput").ap()
    # idx16: host-wrapped gather indices in the SWDGE addressing scheme,
    # [128, b_loc*K/16] int16, replicated into all 8 16-partition blocks.
    # idx16[16r+p, (g, s, cj)] = idx[32g+s, 16cj+p] + 1024*s  (row index into
    # the group's [32*1024, 64] slice of useq)
    idx16 = nc.dram_tensor("idx16", [128, b_loc * K // 16], I16,
                           kind="ExternalInput").ap()
    wq = nc.dram_tensor("wq", [H * D, D], F32, kind="ExternalInput").ap()
    wk = nc.dram_tensor("wk", [H * D, D], F32, kind="ExternalInput").ap()
    wv = nc.dram_tensor("wv", [H * D, D], F32, kind="ExternalInput").ap()
    bq = nc.dram_tensor("bq", [H, D], F32, kind="ExternalInput").ap()
    bv = nc.dram_tensor("bv", [H, D], F32, kind="ExternalInput").ap()
    wo = nc.dram_tensor("wo", [H * D, D], F32, kind="ExternalInput").ap()
    bo = nc.dram_tensor("bo", [D, 1], F32, kind="ExternalInput").ap()
    w1 = nc.dram_tensor("w1", [2 * D, D], F32, kind="ExternalInput").ap()
    b1 = nc.dram_tensor("b1", [D, 1], F32, kind="ExternalInput").ap()
    w2 = nc.dram_tensor("w2", [D, 1], F32, kind="ExternalInput").ap()
    b2 = nc.dram_tensor("b2", [1, 1], F32, kind="ExternalInput").ap()
    logit = nc.dram_tensor("logit", [b_loc, 1], F32, kind="ExternalOutput").ap()

    with tile.TileContext(nc) as tc, \
         tc.tile_pool(name="const", bufs=1) as const, \
         tc.tile_pool(name="stage", bufs=3) as stage, \
         tc.tile_pool(name="gath", bufs=16) as gath_pool, \
         tc.tile_pool(name="gthb", bufs=4) as gthb_pool, \
         tc.tile_pool(name="ptsb", bufs=4) as pt_pool, \
         tc.tile_pool(name="small", bufs=3) as small, \
         tc.tile_pool(name="expsb", bufs=4) as exp_pool, \
         tc.tile_pool(name="rzbsb", bufs=4) as rzb_pool, \
         tc.tile_pool(name="ps_a", bufs=1, space="PSUM") as ps_a, \
         tc.tile_pool(name="scx", bufs=3, space="PSUM") as scx_pool, \
         tc.tile_pool(name="zc", bufs=2, space="PSUM") as zc_pool, \
         tc.tile_pool(name="mhps", bufs=1, space="PSUM") as mh_pool:

        ident = const.tile([128, 128], F32, tag="ident")
        make_identity(nc, ident[:])
        ones16 = const.tile([128, 1], F16, tag="ones16")
        nc.vector.memset(ones16[:], 1.0)
        ones_row = const.tile([1, 128], F32, tag="ones_row")
        nc.vector.memset(ones_row[:], 1.0)

        # ---- index load for dma_gather ----
        # Gather order i = 32*128*g + 128*s + j; the SWDGE ucode reads idx i at
        # idxs[i % 16, i // 16] (int16); queue q's core pair (cpus 2q, 2q+1)
        # reads partitions 32q..32q+32. Host ships the indices pre-wrapped,
        # offset to the group's useq slice, and replicated — one fast DMA here.
        w16_all = const.tile([128, n_grp * 256], I16, tag="w16_all")
        nc.sync.dma_start(out=w16_all[:], in_=idx16[:])
        w16s = [w16_all[:, 256 * g:256 * (g + 1)] for g in range(n_grp)]

        # ---- weight transposes: wqT/wkT/wvT [64, 256] = [e, (h, d)] ----
        wT = {}
        for name, dram in (("wq", wq), ("wk", wk), ("wv", wv)):
            t_sb = const.tile([D, H * D], F32, tag=f"{name}T")
            for t in range(2):
                s = stage.tile([128, D], F32, tag="wstage")
                nc.sync.dma_start(out=s[:], in_=dram[128 * t:128 * (t + 1), :])
                tr = ps_a.tile([D, 128], F32, tag="pa")
                nc.tensor.transpose(tr[:], s[:], ident[:])
                nc.scalar.activation(t_sb[:, 128 * t:128 * (t + 1)], tr[:], AF.Copy)
            wT[name] = t_sb

        # ---- wo_r [64, 256] = [e, (h, f)] (reshaped, not transposed) ----
        wo_r = const.tile([D, H * D], F32, tag="wo_r")
        nc.sync.dma_start(out=wo_r[:].rearrange("p (h f) -> p h f", h=H),
                          in_=wo[:].rearrange("(h e) f -> e h f", h=H))

        # ---- small bias columns ----
        bqcol = const.tile([D, H], F32, tag="bqcol")
        nc.sync.dma_start(out=bqcol[:], in_=bq[:].rearrange("h e -> e h"))
        bvcol = const.tile([D, H], F32, tag="bvcol")
        nc.sync.dma_start(out=bvcol[:], in_=bv[:].rearrange("h e -> e h"))
        bocol = const.tile([D, 1], F32, tag="bocol")
        nc.sync.dma_start(out=bocol[:], in_=bo[:])
        b1col = const.tile([D, 1], F32, tag="b1col")
        nc.sync.dma_start(out=b1col[:], in_=b1[:])
        w1_sb = const.tile([2 * D, D], F32, tag="w1")
        nc.sync.dma_start(out=w1_sb[:], in_=w1[:])
        w2_sb = const.tile([D, 1], F32, tag="w2")
        nc.sync.dma_start(out=w2_sb[:], in_=w2[:])
        b2_sb = const.tile([1, 1], F32, tag="b2")
        nc.sync.dma_start(out=b2_sb[:], in_=b2[:])

        # ---- target transpose: tgtT [64, b_loc]; also xT rows 64:128 ----
        tgtT = const.tile([D, b_loc], F32, tag="tgtT")
        xT = const.tile([128, b_loc], F32, tag="xT")
        for t in range(n_bt):
            r0, r1 = 128 * t, min(128 * (t + 1), b_loc)
            n = r1 - r0
            s = stage.tile([128, D], F32, tag="tstage")
            nc.sync.dma_start(out=s[:n, :], in_=tgt[r0:r1, :])
            tr = ps_a.tile([D, 128], F32, tag="pa")
            nc.tensor.transpose(tr[:, :n], s[:n, :], ident[:n, :n])
            nc.scalar.activation(tgtT[:, r0:r1], tr[:, :n], AF.Copy)
            nc.scalar.activation(xT[D:2 * D, r0:r1], tr[:, :n], AF.Copy)

        # ---- per-head folded matrices ----
        # A_sb[:, 64h:64h+64] = (WQ[h] @ WK[h]^T) / 8 ; c_col = (WK[h] @ bQ[h]) / 8
        A_sb = const.tile([D, H * D], F32, tag="A")
        c_col = const.tile([128, H], F32, tag="c_col")
        c_ps = ps_a.tile([128, H], F32, tag="pa")
        for h in range(H):
            a_ps = ps_a.tile([D, D], F32, tag="pa")
            nc.tensor.matmul(a_ps[:], lhsT=wT["wq"][:, D * h:D * (h + 1)],
                             rhs=wT["wk"][:, D * h:D * (h + 1)], start=True, stop=True)
            nc.scalar.activation(A_sb[:, D * h:D * (h + 1)], a_ps[:], AF.Copy,
                                 scale=0.125)
            nc.tensor.matmul(c_ps[0:D, h:h + 1], lhsT=wT["wk"][:, D * h:D * (h + 1)],
                             rhs=bqcol[:, h:h + 1], start=True, stop=True)
            nc.tensor.matmul(c_ps[D:2 * D, h:h + 1], lhsT=wT["wk"][:, D * h:D * (h + 1)],
                             rhs=bqcol[:, h:h + 1], start=True, stop=True)
        nc.scalar.activation(c_col[:], c_ps[:], AF.Copy, scale=0.125)

        # ---- qk block-diagonal tile [128, 8*n_pair] fp16 ----
        # pair q columns 8q..8q+7: cols 0-3 = even sample heads (rows 0:64),
        # cols 4-7 = odd sample heads (rows 64:128); rest zero.
        qk_bd = const.tile([128, 8 * n_pair], F16, tag="qk_bd")
        nc.vector.memset(qk_bd[:], 0.0)
        tgtT_v = tgtT[:].rearrange("p (s two) -> p s two", two=2)
        qk_v = qk_bd[:].rearrange("p (q c) -> p q c", c=8)
        for h in range(H):
            qk_ps = ps_a.tile([128, n_pair], F32, tag="pa")
            nc.tensor.matmul(qk_ps[0:D, :], lhsT=A_sb[:, D * h:D * (h + 1)],
                             rhs=tgtT_v[:, :, 0], start=True, stop=True)
            nc.tensor.matmul(qk_ps[D:2 * D, :], lhsT=A_sb[:, D * h:D * (h + 1)],
                             rhs=tgtT_v[:, :, 1], start=True, stop=True)
            nc.scalar.activation(qk_v[0:D, :, h], qk_ps[0:D, :], AF.Identity,
                                 bias=c_col[0:D, h:h + 1], scale=1.0)
            nc.scalar.activation(qk_v[D:2 * D, :, 4 + h], qk_ps[D:2 * D, :], AF.Identity,
                                 bias=c_col[D:2 * D, h:h + 1], scale=1.0)

        # ---- G_sb [128, 256] fp16: rows 0:64 / 64:128 both hold G[h] = WV[h] @ WO_h ----
        G_sb = const.tile([128, H * D], F16, tag="G")
        for h in range(H):
            g_ps = ps_a.tile([128, D], F32, tag="pa")
            nc.tensor.matmul(g_ps[0:D, :], lhsT=wT["wv"][:, D * h:D * (h + 1)],
                             rhs=wo_r[:, D * h:D * (h + 1)], start=True, stop=True)
            nc.tensor.matmul(g_ps[D:2 * D, :], lhsT=wT["wv"][:, D * h:D * (h + 1)],
                             rhs=wo_r[:, D * h:D * (h + 1)], start=True, stop=True)
            nc.scalar.activation(G_sb[:, D * h:D * (h + 1)], g_ps[:], AF.Copy)

        # ---- bias0 = sum_h bV[h] @ WO_h + bO, as a [64, 1] column ----
        b0_ps = ps_a.tile([1, D], F32, tag="pa")
        for h in range(H):
            nc.tensor.matmul(b0_ps[:], lhsT=bvcol[:, h:h + 1],
                             rhs=wo_r[:, D * h:D * (h + 1)],
                             start=(h == 0), stop=(h == H - 1))
        b0row = stage.tile([1, D], F32, tag="b0row")
        nc.scalar.activation(b0row[:], b0_ps[:], AF.Copy)
        b0c_ps = ps_a.tile([D, 1], F32, tag="pa")
        nc.tensor.transpose(b0c_ps[:], b0row[:], ident[0:1, 0:1])
        bias0 = const.tile([D, 1], F32, tag="bias0")
        nc.vector.tensor_tensor(out=bias0[:], in0=b0c_ps[:], in1=bocol[:],
                                op=mybir.AluOpType.add)

        # ---- main loop over chunks of 16 samples ----
        ctxn = const.tile([128, 64 * n_chunk], F16, tag="ctxn")
        # ctxn col layout: (chunk c, pair q, col k) -> 64*c + 8*q + k
        ctxn_v = ctxn[:].rearrange("p (c q k) -> p c q k", c=n_chunk, k=8)
        LAG = 2

        def issue_gather(c):
            g = c // 2
            gth = gath_pool.tile([128, 16 * D], F32, tag="gath")
            for hh in range(4):
                eighth = 4 * (c % 2) + hh
                qn = ((4 * c + hh) % 4) if QUEUE_ROT else 0
                nc.gpsimd.dma_gather(
                    out_ap=gth[:, 256 * hh:256 * (hh + 1)].rearrange(
                        "p (s d) -> p s d", d=D),
                    in_ap=useq[32 * L * g:32 * L * (g + 1), :],
                    idxs_ap=w16s[g][:, 32 * eighth:32 * (eighth + 1)],
                    num_idxs=4 * K, num_idxs_reg=4 * K, elem_size=D,
                    single_packet=SINGLE_PACKET, queue_num=qn)
            return gth

        def issue_front(c, gth):
            """fp16 cast + XBAR transpose for chunk c (right after its gather)."""
            gthb = gthb_pool.tile([128, 16 * D], F16, tag="gthb")
            if c % 2:
                nc.scalar.activation(gthb[:], gth[:], AF.Copy)
            else:
                nc.vector.tensor_copy(out=gthb[:], in_=gth[:])
            pt = pt_pool.tile([128, 8 * 128], F16, tag="pt")
            eng = nc.scalar if (c % 2) else nc.sync
            eng.dma_start_transpose(
                out=pt[:].rearrange("p (q j) -> p q j", j=128), in_=gthb[:])
            return gthb, pt

        def issue_compute(c, gthb, pt):
            # scores: per pair lhsT = pT pair [128(s2,d), j], rhs = qk pair cols
            sc_ps = scx_pool.tile([128, 64], F32, tag="scx")
            for q in range(8):
                Q = 8 * c + q
                nc.tensor.matmul(sc_ps[:, 8 * q:8 * (q + 1)],
                                 lhsT=pt[:, 128 * q:128 * (q + 1)],
                                 rhs=qk_bd[:, 8 * Q:8 * (Q + 1)], start=True, stop=True)
            exp_sb = exp_pool.tile([128, 64], F16, tag="exp")
            nc.scalar.activation(exp_sb[:], sc_ps[:], AF.Exp)
            # z row = ones^T @ exp ; rz = 1/z ; rzb = broadcast of rz
            z_ps = zc_pool.tile([1, 64], F32, tag="zc")
            nc.tensor.matmul(z_ps[:], lhsT=ones16[:], rhs=exp_sb[:],
                             start=True, stop=True)
            rz = small.tile([1, 64], F32, tag="rz")
            nc.vector.reciprocal(rz[:], z_ps[:])
            rzb_ps = zc_pool.tile([128, 64], F32, tag="zc")
            nc.tensor.matmul(rzb_ps[:], lhsT=ones_row[:], rhs=rz[:],
                             start=True, stop=True)
            rzb = rzb_pool.tile([128, 64], F32, tag="rzb")
            nc.scalar.activation(rzb[:], rzb_ps[:], AF.Copy)
            # ctx on UNNORMALIZED exp; 1/Z folded in the fp16 convert below.
            ctx_ps = scx_pool.tile([128, 64], F32, tag="scx")
            for q in range(8):
                nc.tensor.matmul(ctx_ps[:, 8 * q:8 * (q + 1)],
                                 lhsT=gthb[:, 128 * q:128 * (q + 1)],
                                 rhs=exp_sb[:, 8 * q:8 * (q + 1)], start=True, stop=True)
            nc.vector.tensor_tensor(out=ctxn[:, 64 * c:64 * (c + 1)], in0=ctx_ps[:],
                                    in1=rzb[:], op=mybir.AluOpType.mult)

        # software pipeline: all gathers issued up front (dedicated buffers, no
        # tile deps between them — they stream on the 4 SWDGE queues), then
        # cast+transpose per chunk with compute LAG chunks behind. The output
        # head mh accumulation runs in two halves so the first half overlaps
        # the second half's gathers.
        mh_e = mh_pool.tile([D, n_pair], F32, tag="mh_e")
        mh_o = mh_pool.tile([D, n_pair], F32, tag="mh_o")
        hc = n_chunk // 2

        def issue_mh(half):
            c0, c1 = half * hc, (half + 1) * hc
            p0, p1 = 8 * half * hc, 8 * (half + 1) * hc
            for h in range(H):
                nc.tensor.matmul(mh_e[:, p0:p1], lhsT=G_sb[0:D, D * h:D * (h + 1)],
                                 rhs=ctxn_v[0:D, c0:c1, :, h],
                                 start=(h == 0), stop=(h == H - 1))
            for h in range(H):
                nc.tensor.matmul(mh_o[:, p0:p1], lhsT=G_sb[D:2 * D, D * h:D * (h + 1)],
                                 rhs=ctxn_v[D:2 * D, c0:c1, :, 4 + h],
                                 start=(h == 0), stop=(h == H - 1))

        gths = {c: issue_gather(c) for c in range(n_chunk)}
        fronts = {}
        for c in range(n_chunk + LAG):
            if c < n_chunk:
                fronts[c] = issue_front(c, gths[c])
            if c >= LAG:
                cc = c - LAG
                issue_compute(cc, *fronts.pop(cc))
                if cc == hc - 1:
                    issue_mh(0)
        issue_mh(1)

        # ---- output head tail: bias + MLP (fp32) ----
        x_v = xT[:].rearrange("p (s two) -> p s two", two=2)
        nc.scalar.activation(x_v[0:D, :, 0], mh_e[:], AF.Identity, bias=bias0[:],
                             scale=1.0)
        nc.scalar.activation(x_v[0:D, :, 1], mh_o[:], AF.Identity, bias=bias0[:],
                             scale=1.0)
        h1_ps = ps_a.tile([D, b_loc], F32, tag="pa")
        nc.tensor.matmul(h1_ps[:], lhsT=w1_sb[:], rhs=xT[:], start=True, stop=True)
        h1_sb = const.tile([D, b_loc], F32, tag="h1")
        nc.scalar.activation(h1_sb[:], h1_ps[:], AF.Relu, bias=b1col[:], scale=1.0)
        lg_ps = ps_a.tile([1, b_loc], F32, tag="pa")
        nc.tensor.matmul(lg_ps[:], lhsT=w2_sb[:], rhs=h1_sb[:], start=True, stop=True)
        lg_sb = const.tile([1, b_loc], F32, tag="lg")
        nc.scalar.activation(lg_sb[:], lg_ps[:], AF.Identity, bias=b2_sb[:], scale=1.0)
        nc.sync.dma_start(out=logit[:], in_=lg_sb[:])

    nc.compile()
    return nc


def make_in_maps(inputs, b_loc=B_LOC, n_cores=N_CORES):
    """Shard full inputs into per-core in_maps (data parallel over batch)."""
    idx = np.asarray(inputs["indices"]).astype(np.int32)
    # SWDGE wrapped layout: idx16[p, (g, s, cj)] = idx[32g+s, 16cj+p] + 1024 s
    # (row index into the group's [32*1024, 64] useq slice), int16, replicated
    # across the 8 16-partition blocks.
    n_grp = b_loc // 32
    idxg = idx.reshape(n_cores * n_grp, 32, K // 16, 16) + \
        (L * np.arange(32, dtype=np.int32))[None, :, None, None]
    idx16_all = np.ascontiguousarray(
        idxg.transpose(3, 0, 1, 2)).astype(np.int16)  # [16, G, 32, 8]
    idx16_all = np.broadcast_to(
        idx16_all[None], (8,) + idx16_all.shape)  # replicate to 128 partitions
    useq = np.ascontiguousarray(np.asarray(inputs["user_seq_emb"], dtype=np.float32))
    tgt = np.ascontiguousarray(np.asarray(inputs["target_emb"], dtype=np.float32)[:, 0, :])
    shared = {
        "wq": np.ascontiguousarray(np.asarray(inputs["WQ"], np.float32).reshape(H * D, D)),
        "wk": np.ascontiguousarray(np.asarray(inputs["WK"], np.float32).reshape(H * D, D)),
        "wv": np.ascontiguousarray(np.asarray(inputs["WV"], np.float32).reshape(H * D, D)),
        "bq": np.ascontiguousarray(np.asarray(inputs["bQ"], np.float32)),
        "bv": np.ascontiguousarray(np.asarray(inputs["bV"], np.float32)),
        "wo": np.ascontiguousarray(np.asarray(inputs["WO"], np.float32)),
        "bo": np.asarray(inputs["bO"], np.float32).reshape(D, 1).copy(),
        "w1": np.ascontiguousarray(np.asarray(inputs["W1"], np.float32)),
        "b1": np.asarray(inputs["b1"], np.float32).reshape(D, 1).copy(),
        "w2": np.ascontiguousarray(np.asarray(inputs["W2"], np.float32)),
        "b2": np.asarray(inputs["b2"], np.float32).reshape(1, 1).copy(),
    }
    in_maps = []
    for c in range(n_cores):
        s = slice(c * b_loc, (c + 1) * b_loc)
        m = dict(shared)
        m["useq"] = useq[s].reshape(b_loc * L, D)
        m["tgt"] = tgt[s]
        m["idx16"] = np.ascontiguousarray(
            idx16_all[:, :, c * n_grp:(c + 1) * n_grp]).reshape(128, b_loc * K // 16)
        in_maps.append(m)
    return in_maps


_NC_CACHE = {}


def kernel(**inputs):
    if B_LOC not in _NC_CACHE:
        _NC_CACHE[B_LOC] = build(B_LOC)
    nc = _NC_CACHE[B_LOC]
    in_maps = make_in_maps(inputs)
    res = run_bass_kernel_spmd(nc, in_maps, core_ids=list(range(N_CORES)))
    return np.concatenate([res.results[c]["logit"] for c in range(N_CORES)], axis=0)


# revision 50
# speedup vs baseline: 1.2454x; 1.0054x over previous
"""Trainium2 Bass kernel for nn_BaseSearchBasedModel (sparse attention).

Math restructuring (exact up to rounding):
  topk   = user_seq_emb[b, indices[b,k]]                      (SWDGE gather)
  scores = topk . (A[h]^T tgt + c[h]) / 8  + const(b,h)       A = WQ WK^T, c = WK bQ
    The const(b,h) term (q.bK) is constant over the softmax axis -> drops out.
  heads  = softmax(scores) @ topk @ WV[h]                     (WV folded after softmax)
  mhta   = sum_h ctx[h] @ G[h] + bias0                        G = WV[h] WO_h
  logit  = MLP(concat(mhta, tgt))

v3: fp16 data path (gathered data, qk, exp, ctx, G) halves PE matmul
passes; per-pair PE transposes replaced by one XBAR DMA transpose per
chunk; softmax z via ones-row matmul + reciprocal + broadcast matmul;
per-chunk gathers rotate SWDGE queues 0-3 so descriptor generation of
chunk c+1 overlaps the SDMA drain of chunk c; output-head G matmuls
batched over all chunks with strided rhs.

Sharding: pure data parallel, batch 2048 -> 8 cores x 256.
"""

import sys

if "/opt/trn_rl_repo" not in sys.path:
    sys.path.insert(0, "/opt/trn_rl_repo")

import numpy as np

import concourse.bass as bass
import concourse.tile as tile
import concourse.mybir as mybir
from concourse import bacc
from concourse.bass_utils import run_bass_kernel_spmd
from concourse.masks import make_identity

F32 = mybir.dt.float32
F16 = mybir.dt.float16
I32 = mybir.dt.int32
I16 = mybir.dt.int16
AF = mybir.ActivationFunctionType

B, L, K, D, H = 2048, 1024, 128, 64, 4
N_CORES = 8
B_LOC = B // N_CORES  # 256

SINGLE_PACKET = False
QUEUE_ROT = True


def build(b_loc=B_LOC):
    """Build the per-core Bass module. b_loc must be a multiple of 32."""
    assert b_loc % 32 == 0
    n_grp = b_loc // 32            # gather idx groups of 32 samples
    n_pair = b_loc // 2            # sample pairs
    n_bt = (b_loc + 127) // 128    # 128-row tiles over the local batch
    n_chunk = b_loc // 16          # 16-sample chunks (half a gather idx group)

    nc = bacc.Bacc("TRN2", target_bir_lowering=False, debug=False, num_devices=N_CORES,
                   num_swdge_queues=4 if QUEUE_ROT else 1,
                   dynamic_dma_scratch_size=65536)

    useq = nc.dram_tensor("useq", [b_loc * L, D], F32, kind="ExternalInput").ap()
    tgt = nc.dram_tensor("tgt", [b_loc, D], F32, kind="ExternalInput").ap()
    # idx16: host-wrapped gather indices in the SWDGE addressing scheme,
    # [128, b_loc*K/16] int16, replicated into all 8 16-partition blocks.
    # idx16[16r+p, (g, s, cj)] = idx[32g+s, 16cj+p] + 1024*s  (row index into
    # the group's [32*1024, 64] slice of useq)
    idx16 = nc.dram_tensor("idx16", [128, b_loc * K // 16], I16,
                           kind="ExternalInput").ap()
    wq = nc.dram_tensor("wq", [H * D, D], F32, kind="ExternalInput").ap()
    wk = nc.dram_tensor("wk", [H * D, D], F32, kind="ExternalInput").ap()
    wv = nc.dram_tensor("wv", [H * D, D], F32, kind="ExternalInput").ap()
    bq = nc.dram_tensor("bq", [H, D], F32, kind="ExternalInput").ap()
    bv = nc.dram_tensor("bv", [H, D], F32, kind="ExternalInput").ap()
    wo = nc.dram_tensor("wo", [H * D, D], F32, kind="ExternalInput").ap()
    bo = nc.dram_tensor("bo", [D, 1], F32, kind="ExternalInput").ap()
    w1 = nc.dram_tensor("w1", [2 * D, D], F32, kind="ExternalInput").ap()
    b1 = nc.dram_tensor("b1", [D, 1], F32, kind="ExternalInput").ap()
    w2 = nc.dram_tensor("w2", [D, 1], F32, kind="ExternalInput").ap()
    b2 = nc.dram_tensor("b2", [1, 1], F32, kind="ExternalInput").ap()
    logit = nc.dram_tensor("logit", [b_loc, 1], F32, kind="ExternalOutput").ap()

    with tile.TileContext(nc) as tc, \
         tc.tile_pool(name="const", bufs=1) as const, \
         tc.tile_pool(name="stage", bufs=3) as stage, \
         tc.tile_pool(name="gath", bufs=16) as gath_pool, \
         tc.tile_pool(name="gthb", bufs=4) as gthb_pool, \
         tc.tile_pool(name="ptsb", bufs=4) as pt_pool, \
         tc.tile_pool(name="small", bufs=3) as small, \
         tc.tile_pool(name="expsb", bufs=4) as exp_pool, \
         tc.tile_pool(name="rzbsb", bufs=4) as rzb_pool, \
         tc.tile_pool(name="ps_a", bufs=1, space="PSUM") as ps_a, \
         tc.tile_pool(name="scx", bufs=3, space="PSUM") as scx_pool, \
         tc.tile_pool(name="zc", bufs=2, space="PSUM") as zc_pool, \
         tc.tile_pool(name="mhps", bufs=1, space="PSUM") as mh_pool:

        ident = const.tile([128, 128], F32, tag="ident")
        make_identity(nc, ident[:])
        ones16 = const.tile([128, 1], F16, tag="ones16")
        nc.vector.memset(ones16[:], 1.0)
        ones_row = const.tile([1, 128], F16, tag="ones_row")
        nc.vector.memset(ones_row[:], 1.0)

        # ---- index load for dma_gather ----
        # Gather order i = 32*128*g + 128*s + j; the SWDGE ucode reads idx i at
        # idxs[i % 16, i // 16] (int16); queue q's core pair (cpus 2q, 2q+1)
        # reads partitions 32q..32q+32. Host ships the indices pre-wrapped,
        # offset to the group's useq slice, and replicated — one fast DMA here.
        w16_all = const.tile([128, n_grp * 256], I16, tag="w16_all")
        for g in range(n_grp):
            nc.sync.dma_start(out=w16_all[:, 256 * g:256 * (g + 1)],
                              in_=idx16[:, 256 * g:256 * (g + 1)])
        w16s = [w16_all[:, 256 * g:256 * (g + 1)] for g in range(n_grp)]

        # ---- weight transposes: wqT/wkT/wvT [64, 256] = [e, (h, d)] ----
        wT = {}
        for name, dram in (("wq", wq), ("wk", wk), ("wv", wv)):
            t_sb = const.tile([D, H * D], F32, tag=f"{name}T")
            for t in range(2):
                s = stage.tile([128, D], F32, tag="wstage")
                nc.sync.dma_start(out=s[:], in_=dram[128 * t:128 * (t + 1), :])
                tr = ps_a.tile([D, 128], F32, tag="pa")
                nc.tensor.transpose(tr[:], s[:], ident[:])
                nc.scalar.activation(t_sb[:, 128 * t:128 * (t + 1)], tr[:], AF.Copy)
            wT[name] = t_sb

        # ---- wo_r [64, 256] = [e, (h, f)] (reshaped, not transposed) ----
        wo_r = const.tile([D, H * D], F32, tag="wo_r")
        nc.sync.dma_start(out=wo_r[:].rearrange("p (h f) -> p h f", h=H),
                          in_=wo[:].rearrange("(h e) f -> e h f", h=H))

        # ---- small bias columns ----
        bqcol = const.tile([D, H], F32, tag="bqcol")
        nc.sync.dma_start(out=bqcol[:], in_=bq[:].rearrange("h e -> e h"))
        bvcol = const.tile([D, H], F32, tag="bvcol")
        nc.sync.dma_start(out=bvcol[:], in_=bv[:].rearrange("h e -> e h"))
        bocol = const.tile([D, 1], F32, tag="bocol")
        nc.sync.dma_start(out=bocol[:], in_=bo[:])
        b1col = const.tile([D, 1], F32, tag="b1col")
        nc.sync.dma_start(out=b1col[:], in_=b1[:])
        w1_st = stage.tile([2 * D, D], F32, tag="w1s")
        nc.sync.dma_start(out=w1_st[:], in_=w1[:])
        w1_sb = const.tile([2 * D, D], F16, tag="w1")
        nc.vector.tensor_copy(out=w1_sb[:], in_=w1_st[:])
        w2_st = stage.tile([D, 1], F32, tag="w2s")
        nc.sync.dma_start(out=w2_st[:], in_=w2[:])
        w2_sb = const.tile([D, 1], F16, tag="w2")
        nc.vector.tensor_copy(out=w2_sb[:], in_=w2_st[:])
        b2_sb = const.tile([1, 1], F32, tag="b2")
        nc.sync.dma_start(out=b2_sb[:], in_=b2[:])

        # ---- target transpose: tgtT [64, b_loc]; also xT rows 64:128 ----
        tgtT = const.tile([D, b_loc], F32, tag="tgtT")
        xT = const.tile([128, b_loc], F16, tag="xT")
        for t in range(n_bt):
            r0, r1 = 128 * t, min(128 * (t + 1), b_loc)
            n = r1 - r0
            s = stage.tile([128, D], F32, tag="tstage")
            nc.sync.dma_start(out=s[:n, :], in_=tgt[r0:r1, :])
            tr = ps_a.tile([D, 128], F32, tag="pa")
            nc.tensor.transpose(tr[:, :n], s[:n, :], ident[:n, :n])
            nc.scalar.activation(tgtT[:, r0:r1], tr[:, :n], AF.Copy)
            nc.scalar.activation(xT[D:2 * D, r0:r1], tr[:, :n], AF.Copy)

        # ---- per-head folded matrices ----
        # A_sb[:, 64h:64h+64] = (WQ[h] @ WK[h]^T) / 8 ; c_col = (WK[h] @ bQ[h]) / 8
        A_sb = const.tile([D, H * D], F32, tag="A")
        c_col = const.tile([128, H], F32, tag="c_col")
        c_ps = ps_a.tile([128, H], F32, tag="pa")
        for h in range(H):
            a_ps = ps_a.tile([D, D], F32, tag="pa")
            nc.tensor.matmul(a_ps[:], lhsT=wT["wq"][:, D * h:D * (h + 1)],
                             rhs=wT["wk"][:, D * h:D * (h + 1)], start=True, stop=True)
            nc.scalar.activation(A_sb[:, D * h:D * (h + 1)], a_ps[:], AF.Copy,
                                 scale=0.125)
            nc.tensor.matmul(c_ps[0:D, h:h + 1], lhsT=wT["wk"][:, D * h:D * (h + 1)],
                             rhs=bqcol[:, h:h + 1], start=True, stop=True)
            nc.tensor.matmul(c_ps[D:2 * D, h:h + 1], lhsT=wT["wk"][:, D * h:D * (h + 1)],
                             rhs=bqcol[:, h:h + 1], start=True, stop=True)
        nc.scalar.activation(c_col[:], c_ps[:], AF.Copy, scale=0.125)

        # ---- qk block-diagonal tile [128, 8*n_pair] fp16 ----
        # pair q columns 8q..8q+7: cols 0-3 = even sample heads (rows 0:64),
        # cols 4-7 = odd sample heads (rows 64:128); rest zero.
        qk_bd = const.tile([128, 8 * n_pair], F16, tag="qk_bd")
        nc.vector.memset(qk_bd[:], 0.0)
        tgtT_v = tgtT[:].rearrange("p (s two) -> p s two", two=2)
        qk_v = qk_bd[:].rearrange("p (q c) -> p q c", c=8)
        for h in range(H):
            qk_ps = ps_a.tile([128, n_pair], F32, tag="pa")
            nc.tensor.matmul(qk_ps[0:D, :], lhsT=A_sb[:, D * h:D * (h + 1)],
                             rhs=tgtT_v[:, :, 0], start=True, stop=True)
            nc.tensor.matmul(qk_ps[D:2 * D, :], lhsT=A_sb[:, D * h:D * (h + 1)],
                             rhs=tgtT_v[:, :, 1], start=True, stop=True)
            nc.scalar.activation(qk_v[0:D, :, h], qk_ps[0:D, :], AF.Identity,
                                 bias=c_col[0:D, h:h + 1], scale=1.0)
            nc.scalar.activation(qk_v[D:2 * D, :, 4 + h], qk_ps[D:2 * D, :], AF.Identity,
                                 bias=c_col[D:2 * D, h:h + 1], scale=1.0)

        # ---- G_sb [128, 256] fp16: rows 0:64 / 64:128 both hold G[h] = WV[h] @ WO_h ----
        G_sb = const.tile([128, H * D], F16, tag="G")
        for h in range(H):
            g_ps = ps_a.tile([128, D], F32, tag="pa")
            nc.tensor.matmul(g_ps[0:D, :], lhsT=wT["wv"][:, D * h:D * (h + 1)],
                             rhs=wo_r[:, D * h:D * (h + 1)], start=True, stop=True)
            nc.tensor.matmul(g_ps[D:2 * D, :], lhsT=wT["wv"][:, D * h:D * (h + 1)],
                             rhs=wo_r[:, D * h:D * (h + 1)], start=True, stop=True)
            nc.scalar.activation(G_sb[:, D * h:D * (h + 1)], g_ps[:], AF.Copy)

        # ---- bias0 = sum_h bV[h] @ WO_h + bO, as a [64, 1] column ----
        b0_ps = ps_a.tile([1, D], F32, tag="pa")
        for h in range(H):
            nc.tensor.matmul(b0_ps[:], lhsT=bvcol[:, h:h + 1],
                             rhs=wo_r[:, D * h:D * (h + 1)],
                             start=(h == 0), stop=(h == H - 1))
        b0row = stage.tile([1, D], F32, tag="b0row")
        nc.scalar.activation(b0row[:], b0_ps[:], AF.Copy)
        b0c_ps = ps_a.tile([D, 1], F32, tag="pa")
        nc.tensor.transpose(b0c_ps[:], b0row[:], ident[0:1, 0:1])
        bias0 = const.tile([D, 1], F32, tag="bias0")
        nc.vector.tensor_tensor(out=bias0[:], in0=b0c_ps[:], in1=bocol[:],
                                op=mybir.AluOpType.add)

        # ---- main loop over chunks of 16 samples ----
        ctxn = const.tile([128, 64 * n_chunk], F16, tag="ctxn")
        # ctxn col layout: (chunk c, pair q, col k) -> 64*c + 8*q + k
        ctxn_v = ctxn[:].rearrange("p (c q k) -> p c q k", c=n_chunk, k=8)
        LAG = 2

        def issue_gather(c):
            g = c // 2
            gth = gath_pool.tile([128, 16 * D], F32, tag="gath")
            for hh in range(4):
                eighth = 4 * (c % 2) + hh
                qn = ((4 * c + hh) % 4) if QUEUE_ROT else 0
                nc.gpsimd.dma_gather(
                    out_ap=gth[:, 256 * hh:256 * (hh + 1)].rearrange(
                        "p (s d) -> p s d", d=D),
                    in_ap=useq[32 * L * g:32 * L * (g + 1), :],
                    idxs_ap=w16s[g][:, 32 * eighth:32 * (eighth + 1)],
                    num_idxs=4 * K, num_idxs_reg=4 * K, elem_size=D,
                    single_packet=SINGLE_PACKET, queue_num=qn)
            return gth

        def issue_front(c, gth):
            """fp16 cast + XBAR transpose for chunk c (right after its gather)."""
            gthb = gthb_pool.tile([128, 16 * D], F16, tag="gthb")
            if c % 2:
                nc.scalar.activation(gthb[:], gth[:], AF.Copy)
            else:
                nc.vector.tensor_copy(out=gthb[:], in_=gth[:])
            pt = pt_pool.tile([128, 8 * 128], F16, tag="pt")
            eng = nc.scalar if (c % 2) else nc.sync
            eng.dma_start_transpose(
                out=pt[:].rearrange("p (q j) -> p q j", j=128), in_=gthb[:])
            return gthb, pt

        def issue_compute(c, gthb, pt):
            # scores: per pair lhsT = pT pair [128(s2,d), j], rhs = qk pair cols
            sc_ps = scx_pool.tile([128, 64], F32, tag="scx")
            for q in range(8):
                Q = 8 * c + q
                nc.tensor.matmul(sc_ps[:, 8 * q:8 * (q + 1)],
                                 lhsT=pt[:, 128 * q:128 * (q + 1)],
                                 rhs=qk_bd[:, 8 * Q:8 * (Q + 1)], start=True, stop=True)
            exp_sb = exp_pool.tile([128, 64], F16, tag="exp")
            nc.scalar.activation(exp_sb[:], sc_ps[:], AF.Exp)
            # z row = ones^T @ exp ; rz = 1/z ; rzb = broadcast of rz
            z_ps = zc_pool.tile([1, 64], F32, tag="zc")
            nc.tensor.matmul(z_ps[:], lhsT=ones16[:], rhs=exp_sb[:],
                             start=True, stop=True)
            rz = small.tile([1, 64], F16, tag="rz")
            with nc.allow_low_precision(reason="1/Z in fp16 is ample for 2e-2 gate"):
                nc.vector.reciprocal(rz[:], z_ps[:])
            rzb_ps = zc_pool.tile([128, 64], F32, tag="zc")
            nc.tensor.matmul(rzb_ps[:], lhsT=ones_row[:], rhs=rz[:],
                             start=True, stop=True)
            rzb = rzb_pool.tile([128, 64], F32, tag="rzb")
            nc.scalar.activation(rzb[:], rzb_ps[:], AF.Copy)
            # ctx on UNNORMALIZED exp; 1/Z folded in the fp16 convert below.
            ctx_ps = scx_pool.tile([128, 64], F32, tag="scx")
            for q in range(8):
                nc.tensor.matmul(ctx_ps[:, 8 * q:8 * (q + 1)],
                                 lhsT=gthb[:, 128 * q:128 * (q + 1)],
                                 rhs=exp_sb[:, 8 * q:8 * (q + 1)], start=True, stop=True)
            nc.vector.tensor_tensor(out=ctxn[:, 64 * c:64 * (c + 1)], in0=ctx_ps[:],
                                    in1=rzb[:], op=mybir.AluOpType.mult)

        # software pipeline: all gathers issued up front (dedicated buffers, no
        # tile deps between them — they stream on the 4 SWDGE queues), then
        # cast+transpose per chunk with compute LAG chunks behind. The output
        # head mh accumulation runs in two halves so the first half overlaps
        # the second half's gathers.
        mh_e = mh_pool.tile([D, n_pair], F32, tag="mh_e")
        mh_o = mh_pool.tile([D, n_pair], F32, tag="mh_o")
        hc = n_chunk // 2

        def issue_mh(half):
            c0, c1 = half * hc, (half + 1) * hc
            p0, p1 = 8 * half * hc, 8 * (half + 1) * hc
            for h in range(H):
                nc.tensor.matmul(mh_e[:, p0:p1], lhsT=G_sb[0:D, D * h:D * (h + 1)],
                                 rhs=ctxn_v[0:D, c0:c1, :, h],
                                 start=(h == 0), stop=(h == H - 1))
            for h in range(H):
                nc.tensor.matmul(mh_o[:, p0:p1], lhsT=G_sb[D:2 * D, D * h:D * (h + 1)],
                                 rhs=ctxn_v[D:2 * D, c0:c1, :, 4 + h],
                                 start=(h == 0), stop=(h == H - 1))

        gths = {c: issue_gather(c) for c in range(n_chunk)}
        fronts = {}
        for c in range(n_chunk + LAG):
            if c < n_chunk:
                fronts[c] = issue_front(c, gths[c])
            if c >= LAG:
                cc = c - LAG
                issue_compute(cc, *fronts.pop(cc))
                if cc == hc - 1:
                    issue_mh(0)
        issue_mh(1)

        # ---- output head tail: bias + MLP (fp32) ----
        x_v = xT[:].rearrange("p (s two) -> p s two", two=2)
        nc.scalar.activation(x_v[0:D, :, 0], mh_e[:], AF.Identity, bias=bias0[:],
                             scale=1.0)
        nc.scalar.activation(x_v[0:D, :, 1], mh_o[:], AF.Identity, bias=bias0[:],
                             scale=1.0)
        h1_ps = ps_a.tile([D, b_loc], F32, tag="pa")
        nc.tensor.matmul(h1_ps[:], lhsT=w1_sb[:], rhs=xT[:], start=True, stop=True)
        h1_sb = const.tile([D, b_loc], F16, tag="h1")
        nc.scalar.activation(h1_sb[:], h1_ps[:], AF.Relu, bias=b1col[:], scale=1.0)
        lg_ps = ps_a.tile([1, b_loc], F32, tag="pa")
        nc.tensor.matmul(lg_ps[:], lhsT=w2_sb[:], rhs=h1_sb[:], start=True, stop=True)
        lg_sb = const.tile([1, b_loc], F32, tag="lg")
        nc.scalar.activation(lg_sb[:], lg_ps[:], AF.Identity, bias=b2_sb[:], scale=1.0)
        nc.sync.dma_start(out=logit[:], in_=lg_sb[:])

    nc.compile()
    return nc


def make_in_maps(inputs, b_loc=B_LOC, n_cores=N_CORES):
    """Shard full inputs into per-core in_maps (data parallel over batch)."""
    idx = np.asarray(inputs["indices"]).astype(np.int32)
    # SWDGE wrapped layout: idx16[p, (g, s, cj)] = idx[32g+s, 16cj+p] + 1024 s
    # (row index into the group's [32*1024, 64] useq slice), int16, replicated
    # across the 8 16-partition blocks.
    n_grp = b_loc // 32
    idxg = idx.reshape(n_cores * n_grp, 32, K // 16, 16) + \
        (L * np.arange(32, dtype=np.int32))[None, :, None, None]
    idx16_all = np.ascontiguousarray(
        idxg.transpose(3, 0, 1, 2)).astype(np.int16)  # [16, G, 32, 8]
    idx16_all = np.broadcast_to(
        idx16_all[None], (8,) + idx16_all.shape)  # replicate to 128 partitions
    useq = np.ascontiguousarray(np.asarray(inputs["user_seq_emb"], dtype=np.float32))
    tgt = np.ascontiguousarray(np.asarray(inputs["target_emb"], dtype=np.float32)[:, 0, :])
    shared = {
        "wq": np.ascontiguousarray(np.asarray(inputs["WQ"], np.float32).reshape(H * D, D)),
        "wk": np.ascontiguousarray(np.asarray(inputs["WK"], np.float32).reshape(H * D, D)),
        "wv": np.ascontiguousarray(np.asarray(inputs["WV"], np.float32).reshape(H * D, D)),
        "bq": np.ascontiguousarray(np.asarray(inputs["bQ"], np.float32)),
        "bv": np.ascontiguousarray(np.asarray(inputs["bV"], np.float32)),
        "wo": np.ascontiguousarray(np.asarray(inputs["WO"], np.float32)),
        "bo": np.asarray(inputs["bO"], np.float32).reshape(D, 1).copy(),
        "w1": np.ascontiguousarray(np.asarray(inputs["W1"], np.float32)),
        "b1": np.asarray(inputs["b1"], np.float32).reshape(D, 1).copy(),
        "w2": np.ascontiguousarray(np.asarray(inputs["W2"], np.float32)),
        "b2": np.asarray(inputs["b2"], np.float32).reshape(1, 1).copy(),
    }
    in_maps = []
    for c in range(n_cores):
        s = slice(c * b_loc, (c + 1) * b_loc)
        m = dict(shared)
        m["useq"] = useq[s].reshape(b_loc * L, D)
        m["tgt"] = tgt[s]
        m["idx16"] = np.ascontiguousarray(
            idx16_all[:, :, c * n_grp:(c + 1) * n_grp]).reshape(128, b_loc * K // 16)
        in_maps.append(m)
    return in_maps


_NC_CACHE = {}


def kernel(**inputs):
    if B_LOC not in _NC_CACHE:
        _NC_CACHE[B_LOC] = build(B_LOC)
    nc = _NC_CACHE[B_LOC]
    in_maps = make_in_maps(inputs)
    res = run_bass_kernel_spmd(nc, in_maps, core_ids=list(range(N_CORES)))
    return np.concatenate([res.results[c]["logit"] for c in range(N_CORES)], axis=0)


# revision 51
# speedup vs baseline: 1.4598x; 1.1721x over previous
"""Trainium2 Bass kernel for nn_BaseSearchBasedModel (sparse attention).

Math restructuring (exact up to rounding):
  topk   = user_seq_emb[b, indices[b,k]]                      (SWDGE gather)
  scores = topk . (A[h]^T tgt + c[h]) / 8  + const(b,h)       A = WQ WK^T, c = WK bQ
    The const(b,h) term (q.bK) is constant over the softmax axis -> drops out.
  heads  = softmax(scores) @ topk @ WV[h]                     (WV folded after softmax)
  mhta   = sum_h ctx[h] @ G[h] + bias0                        G = WV[h] WO_h
  logit  = MLP(concat(mhta, tgt))

v3: fp16 data path (gathered data, qk, exp, ctx, G) halves PE matmul
passes; per-pair PE transposes replaced by one XBAR DMA transpose per
chunk; softmax z via ones-row matmul + reciprocal + broadcast matmul;
per-chunk gathers rotate SWDGE queues 0-3 so descriptor generation of
chunk c+1 overlaps the SDMA drain of chunk c; output-head G matmuls
batched over all chunks with strided rhs.

Sharding: pure data parallel, batch 2048 -> 8 cores x 256.
"""

import sys

if "/opt/trn_rl_repo" not in sys.path:
    sys.path.insert(0, "/opt/trn_rl_repo")

import numpy as np

import concourse.bass as bass
import concourse.tile as tile
import concourse.mybir as mybir
from concourse import bacc
from concourse.bass_utils import run_bass_kernel_spmd
from concourse.masks import make_identity

F32 = mybir.dt.float32
F16 = mybir.dt.float16
I32 = mybir.dt.int32
I16 = mybir.dt.int16
AF = mybir.ActivationFunctionType

B, L, K, D, H = 2048, 1024, 128, 64, 4
N_CORES = 8
B_LOC = B // N_CORES  # 256

SINGLE_PACKET = False
QUEUE_ROT = True


def build(b_loc=B_LOC):
    """Build the per-core Bass module. b_loc must be a multiple of 32."""
    assert b_loc % 32 == 0
    n_grp = b_loc // 32            # gather idx groups of 32 samples
    n_pair = b_loc // 2            # sample pairs
    n_bt = (b_loc + 127) // 128    # 128-row tiles over the local batch
    n_chunk = b_loc // 16          # 16-sample chunks (half a gather idx group)

    nc = bacc.Bacc("TRN2", target_bir_lowering=False, debug=False, num_devices=N_CORES,
                   num_swdge_queues=4 if QUEUE_ROT else 1,
                   dynamic_dma_scratch_size=65536)

    useq = nc.dram_tensor("useq", [b_loc * L, D], F32, kind="ExternalInput").ap()
    tgt = nc.dram_tensor("tgt", [b_loc, D], F32, kind="ExternalInput").ap()
    # idx16: host-wrapped gather indices in the SWDGE addressing scheme,
    # [128, b_loc*K/16] int16, replicated into all 8 16-partition blocks.
    # idx16[16r+p, (g, s, cj)] = idx[32g+s, 16cj+p] + 1024*s  (row index into
    # the group's [32*1024, 64] slice of useq)
    idx16 = nc.dram_tensor("idx16", [128, b_loc * K // 16], I16,
                           kind="ExternalInput").ap()
    wq = nc.dram_tensor("wq", [H * D, D], F32, kind="ExternalInput").ap()
    wk = nc.dram_tensor("wk", [H * D, D], F32, kind="ExternalInput").ap()
    wv = nc.dram_tensor("wv", [H * D, D], F32, kind="ExternalInput").ap()
    bq = nc.dram_tensor("bq", [H, D], F32, kind="ExternalInput").ap()
    bv = nc.dram_tensor("bv", [H, D], F32, kind="ExternalInput").ap()
    wo = nc.dram_tensor("wo", [H * D, D], F32, kind="ExternalInput").ap()
    bo = nc.dram_tensor("bo", [D, 1], F32, kind="ExternalInput").ap()
    w1 = nc.dram_tensor("w1", [2 * D, D], F32, kind="ExternalInput").ap()
    b1 = nc.dram_tensor("b1", [D, 1], F32, kind="ExternalInput").ap()
    w2 = nc.dram_tensor("w2", [D, 1], F32, kind="ExternalInput").ap()
    b2 = nc.dram_tensor("b2", [1, 1], F32, kind="ExternalInput").ap()
    logit = nc.dram_tensor("logit", [b_loc, 1], F32, kind="ExternalOutput").ap()

    with tile.TileContext(nc) as tc, \
         tc.tile_pool(name="const", bufs=1) as const, \
         tc.tile_pool(name="stage", bufs=3) as stage, \
         tc.tile_pool(name="gath", bufs=16) as gath_pool, \
         tc.tile_pool(name="gthb", bufs=4) as gthb_pool, \
         tc.tile_pool(name="ptsb", bufs=4) as pt_pool, \
         tc.tile_pool(name="small", bufs=3) as small, \
         tc.tile_pool(name="expsb", bufs=4) as exp_pool, \
         tc.tile_pool(name="rzbsb", bufs=4) as rzb_pool, \
         tc.tile_pool(name="ps_a", bufs=1, space="PSUM") as ps_a, \
         tc.tile_pool(name="scx", bufs=3, space="PSUM") as scx_pool, \
         tc.tile_pool(name="zc", bufs=2, space="PSUM") as zc_pool, \
         tc.tile_pool(name="mhps", bufs=1, space="PSUM") as mh_pool:

        ident = const.tile([128, 128], F32, tag="ident")
        make_identity(nc, ident[:])
        ones16 = const.tile([128, 1], F16, tag="ones16")
        nc.vector.memset(ones16[:], 1.0)
        ones_row = const.tile([1, 128], F16, tag="ones_row")
        nc.vector.memset(ones_row[:], 1.0)

        # ---- index load for dma_gather ----
        # Gather order i = 32*128*g + 128*s + j; the SWDGE ucode reads idx i at
        # idxs[i % 16, i // 16] (int16); queue q's core pair (cpus 2q, 2q+1)
        # reads partitions 32q..32q+32. Host ships the indices pre-wrapped,
        # offset to the group's useq slice, and replicated — one fast DMA here.
        w16_all = const.tile([128, n_grp * 256], I16, tag="w16_all")
        for g in range(n_grp):
            nc.sync.dma_start(out=w16_all[:, 256 * g:256 * (g + 1)],
                              in_=idx16[:, 256 * g:256 * (g + 1)])
        w16s = [w16_all[:, 256 * g:256 * (g + 1)] for g in range(n_grp)]

        # ---- weight transposes: wqT/wkT/wvT [64, 256] = [e, (h, d)] ----
        wT = {}
        for name, dram in (("wq", wq), ("wk", wk), ("wv", wv)):
            t_sb = const.tile([D, H * D], F32, tag=f"{name}T")
            for t in range(2):
                s = stage.tile([128, D], F32, tag="wstage")
                nc.sync.dma_start(out=s[:], in_=dram[128 * t:128 * (t + 1), :])
                tr = ps_a.tile([D, 128], F32, tag="pa")
                nc.tensor.transpose(tr[:], s[:], ident[:])
                nc.scalar.activation(t_sb[:, 128 * t:128 * (t + 1)], tr[:], AF.Copy)
            wT[name] = t_sb

        # ---- wo_r [64, 256] = [e, (h, f)] (reshaped, not transposed) ----
        wo_r = const.tile([D, H * D], F32, tag="wo_r")
        nc.sync.dma_start(out=wo_r[:].rearrange("p (h f) -> p h f", h=H),
                          in_=wo[:].rearrange("(h e) f -> e h f", h=H))

        # ---- small bias columns ----
        bqcol = const.tile([D, H], F32, tag="bqcol")
        nc.sync.dma_start(out=bqcol[:], in_=bq[:].rearrange("h e -> e h"))
        bvcol = const.tile([D, H], F32, tag="bvcol")
        nc.sync.dma_start(out=bvcol[:], in_=bv[:].rearrange("h e -> e h"))
        bocol = const.tile([D, 1], F32, tag="bocol")
        nc.sync.dma_start(out=bocol[:], in_=bo[:])
        b1col = const.tile([D, 1], F32, tag="b1col")
        nc.sync.dma_start(out=b1col[:], in_=b1[:])
        w1_st = stage.tile([2 * D, D], F32, tag="w1s")
        nc.sync.dma_start(out=w1_st[:], in_=w1[:])
        w1_sb = const.tile([2 * D, D], F16, tag="w1")
        nc.vector.tensor_copy(out=w1_sb[:], in_=w1_st[:])
        w2_st = stage.tile([D, 1], F32, tag="w2s")
        nc.sync.dma_start(out=w2_st[:], in_=w2[:])
        w2_sb = const.tile([D, 1], F16, tag="w2")
        nc.vector.tensor_copy(out=w2_sb[:], in_=w2_st[:])
        b2_sb = const.tile([1, 1], F32, tag="b2")
        nc.sync.dma_start(out=b2_sb[:], in_=b2[:])

        # ---- target transpose: tgtT [64, b_loc]; also xT rows 64:128 ----
        tgtT = const.tile([D, b_loc], F32, tag="tgtT")
        xT = const.tile([128, b_loc], F16, tag="xT")
        for t in range(n_bt):
            r0, r1 = 128 * t, min(128 * (t + 1), b_loc)
            n = r1 - r0
            s = stage.tile([128, D], F32, tag="tstage")
            nc.sync.dma_start(out=s[:n, :], in_=tgt[r0:r1, :])
            tr = ps_a.tile([D, 128], F32, tag="pa")
            nc.tensor.transpose(tr[:, :n], s[:n, :], ident[:n, :n])
            nc.scalar.activation(tgtT[:, r0:r1], tr[:, :n], AF.Copy)
            nc.scalar.activation(xT[D:2 * D, r0:r1], tr[:, :n], AF.Copy)

        # ---- per-head folded matrices ----
        # A_sb[:, 64h:64h+64] = (WQ[h] @ WK[h]^T) / 8 ; c_col = (WK[h] @ bQ[h]) / 8
        A_sb = const.tile([D, H * D], F32, tag="A")
        c_col = const.tile([128, H], F32, tag="c_col")
        c_ps = ps_a.tile([128, H], F32, tag="pa")
        for h in range(H):
            a_ps = ps_a.tile([D, D], F32, tag="pa")
            nc.tensor.matmul(a_ps[:], lhsT=wT["wq"][:, D * h:D * (h + 1)],
                             rhs=wT["wk"][:, D * h:D * (h + 1)], start=True, stop=True)
            nc.scalar.activation(A_sb[:, D * h:D * (h + 1)], a_ps[:], AF.Copy,
                                 scale=0.125)
            nc.tensor.matmul(c_ps[0:D, h:h + 1], lhsT=wT["wk"][:, D * h:D * (h + 1)],
                             rhs=bqcol[:, h:h + 1], start=True, stop=True)
            nc.tensor.matmul(c_ps[D:2 * D, h:h + 1], lhsT=wT["wk"][:, D * h:D * (h + 1)],
                             rhs=bqcol[:, h:h + 1], start=True, stop=True)
        nc.scalar.activation(c_col[:], c_ps[:], AF.Copy, scale=0.125)

        # ---- qk block-diagonal tile [128, 8*n_pair] fp16 ----
        # pair q columns 8q..8q+7: cols 0-3 = even sample heads (rows 0:64),
        # cols 4-7 = odd sample heads (rows 64:128); rest zero.
        qk_bd = const.tile([128, 8 * n_pair], F16, tag="qk_bd")
        nc.vector.memset(qk_bd[:], 0.0)
        tgtT_v = tgtT[:].rearrange("p (s two) -> p s two", two=2)
        qk_v = qk_bd[:].rearrange("p (q c) -> p q c", c=8)
        for h in range(H):
            qk_ps = ps_a.tile([128, n_pair], F32, tag="pa")
            nc.tensor.matmul(qk_ps[0:D, :], lhsT=A_sb[:, D * h:D * (h + 1)],
                             rhs=tgtT_v[:, :, 0], start=True, stop=True)
            nc.tensor.matmul(qk_ps[D:2 * D, :], lhsT=A_sb[:, D * h:D * (h + 1)],
                             rhs=tgtT_v[:, :, 1], start=True, stop=True)
            nc.scalar.activation(qk_v[0:D, :, h], qk_ps[0:D, :], AF.Identity,
                                 bias=c_col[0:D, h:h + 1], scale=1.0)
            nc.scalar.activation(qk_v[D:2 * D, :, 4 + h], qk_ps[D:2 * D, :], AF.Identity,
                                 bias=c_col[D:2 * D, h:h + 1], scale=1.0)

        # ---- G_sb [128, 256] fp16: rows 0:64 / 64:128 both hold G[h] = WV[h] @ WO_h ----
        G_sb = const.tile([128, H * D], F16, tag="G")
        for h in range(H):
            g_ps = ps_a.tile([128, D], F32, tag="pa")
            nc.tensor.matmul(g_ps[0:D, :], lhsT=wT["wv"][:, D * h:D * (h + 1)],
                             rhs=wo_r[:, D * h:D * (h + 1)], start=True, stop=True)
            nc.tensor.matmul(g_ps[D:2 * D, :], lhsT=wT["wv"][:, D * h:D * (h + 1)],
                             rhs=wo_r[:, D * h:D * (h + 1)], start=True, stop=True)
            nc.scalar.activation(G_sb[:, D * h:D * (h + 1)], g_ps[:], AF.Copy)

        # ---- bias0 = sum_h bV[h] @ WO_h + bO, as a [64, 1] column ----
        b0_ps = ps_a.tile([1, D], F32, tag="pa")
        for h in range(H):
            nc.tensor.matmul(b0_ps[:], lhsT=bvcol[:, h:h + 1],
                             rhs=wo_r[:, D * h:D * (h + 1)],
                             start=(h == 0), stop=(h == H - 1))
        b0row = stage.tile([1, D], F32, tag="b0row")
        nc.scalar.activation(b0row[:], b0_ps[:], AF.Copy)
        b0c_ps = ps_a.tile([D, 1], F32, tag="pa")
        nc.tensor.transpose(b0c_ps[:], b0row[:], ident[0:1, 0:1])
        bias0 = const.tile([D, 1], F32, tag="bias0")
        nc.vector.tensor_tensor(out=bias0[:], in0=b0c_ps[:], in1=bocol[:],
                                op=mybir.AluOpType.add)

        # ---- main loop over chunks of 16 samples ----
        ctxn = const.tile([128, 64 * n_chunk], F16, tag="ctxn")
        # ctxn col layout: (chunk c, pair q, col k) -> 64*c + 8*q + k
        ctxn_v = ctxn[:].rearrange("p (c q k) -> p c q k", c=n_chunk, k=8)
        LAG = 2

        def issue_gather(c):
            g = c // 2
            gth = gath_pool.tile([128, 16 * D], F32, tag="gath")
            for hh in range(4):
                eighth = 4 * (c % 2) + hh
                qn = ((4 * c + hh) % 4) if QUEUE_ROT else 0
                nc.gpsimd.dma_gather(
                    out_ap=gth[:, 256 * hh:256 * (hh + 1)].rearrange(
                        "p (s d) -> p s d", d=D),
                    in_ap=useq[32 * L * g:32 * L * (g + 1), :],
                    idxs_ap=w16s[g][:, 32 * eighth:32 * (eighth + 1)],
                    num_idxs=4 * K, num_idxs_reg=4 * K, elem_size=D,
                    single_packet=SINGLE_PACKET, queue_num=qn)
            return gth

        def issue_front(c, gth):
            """fp16 cast + XBAR transpose for chunk c (right after its gather)."""
            gthb = gthb_pool.tile([128, 16 * D], F16, tag="gthb")
            if c % 2:
                nc.scalar.activation(gthb[:], gth[:], AF.Copy)
            else:
                nc.vector.tensor_copy(out=gthb[:], in_=gth[:])
            pt = pt_pool.tile([128, 8 * 128], F16, tag="pt")
            eng = nc.scalar if (c % 2) else nc.sync
            eng.dma_start_transpose(
                out=pt[:].rearrange("p (q j) -> p q j", j=128), in_=gthb[:])
            return gthb, pt

        def issue_compute(c, gthb, pt):
            # scores: per pair lhsT = pT pair [128(s2,d), j], rhs = qk pair cols
            sc_ps = scx_pool.tile([128, 64], F32, tag="scx")
            for q in range(8):
                Q = 8 * c + q
                nc.tensor.matmul(sc_ps[:, 8 * q:8 * (q + 1)],
                                 lhsT=pt[:, 128 * q:128 * (q + 1)],
                                 rhs=qk_bd[:, 8 * Q:8 * (Q + 1)], start=True, stop=True)
            exp_sb = exp_pool.tile([128, 64], F16, tag="exp")
            nc.scalar.activation(exp_sb[:], sc_ps[:], AF.Exp)
            # z row = ones^T @ exp ; rz = 1/z ; rzb = broadcast of rz
            z_ps = zc_pool.tile([1, 64], F32, tag="zc")
            nc.tensor.matmul(z_ps[:], lhsT=ones16[:], rhs=exp_sb[:],
                             start=True, stop=True)
            rz = small.tile([1, 64], F16, tag="rz")
            with nc.allow_low_precision(reason="1/Z in fp16 is ample for 2e-2 gate"):
                nc.vector.reciprocal(rz[:], z_ps[:])
            rzb_ps = zc_pool.tile([128, 64], F32, tag="zc")
            nc.tensor.matmul(rzb_ps[:], lhsT=ones_row[:], rhs=rz[:],
                             start=True, stop=True)
            rzb = rzb_pool.tile([128, 64], F32, tag="rzb")
            nc.scalar.activation(rzb[:], rzb_ps[:], AF.Copy)
            # ctx on UNNORMALIZED exp; 1/Z folded in the fp16 convert below.
            ctx_ps = scx_pool.tile([128, 64], F32, tag="scx")
            for q in range(8):
                nc.tensor.matmul(ctx_ps[:, 8 * q:8 * (q + 1)],
                                 lhsT=gthb[:, 128 * q:128 * (q + 1)],
                                 rhs=exp_sb[:, 8 * q:8 * (q + 1)], start=True, stop=True)
            nc.vector.tensor_tensor(out=ctxn[:, 64 * c:64 * (c + 1)], in0=ctx_ps[:],
                                    in1=rzb[:], op=mybir.AluOpType.mult)

        # software pipeline: all gathers issued up front (dedicated buffers, no
        # tile deps between them — they stream on the 4 SWDGE queues), then
        # cast+transpose per chunk with compute LAG chunks behind. The output
        # head mh accumulation runs in two halves so the first half overlaps
        # the second half's gathers.
        mh_e = mh_pool.tile([D, n_pair], F32, tag="mh_e")
        mh_o = mh_pool.tile([D, n_pair], F32, tag="mh_o")
        hc = n_chunk // 2

        def issue_mh(half):
            c0, c1 = half * hc, (half + 1) * hc
            p0, p1 = 8 * half * hc, 8 * (half + 1) * hc
            for h in range(H):
                nc.tensor.matmul(mh_e[:, p0:p1], lhsT=G_sb[0:D, D * h:D * (h + 1)],
                                 rhs=ctxn_v[0:D, c0:c1, :, h],
                                 start=(h == 0), stop=(h == H - 1))
            for h in range(H):
                nc.tensor.matmul(mh_o[:, p0:p1], lhsT=G_sb[D:2 * D, D * h:D * (h + 1)],
                                 rhs=ctxn_v[D:2 * D, c0:c1, :, 4 + h],
                                 start=(h == 0), stop=(h == H - 1))

        gths = {c: issue_gather(c) for c in range(n_chunk)}
        fronts = {}
        for c in range(n_chunk + LAG):
            if c < n_chunk:
                fronts[c] = issue_front(c, gths[c])
            if c >= LAG:
                cc = c - LAG
                issue_compute(cc, *fronts.pop(cc))
                if cc == hc - 1:
                    issue_mh(0)
        issue_mh(1)

        # ---- output head tail: bias + MLP (fp32) ----
        x_v = xT[:].rearrange("p (s two) -> p s two", two=2)
        nc.scalar.activation(x_v[0:D, :, 0], mh_e[:], AF.Identity, bias=bias0[:],
                             scale=1.0)
        nc.scalar.activation(x_v[0:D, :, 1], mh_o[:], AF.Identity, bias=bias0[:],
                             scale=1.0)
        h1_ps = ps_a.tile([D, b_loc], F32, tag="pa")
        nc.tensor.matmul(h1_ps[:], lhsT=w1_sb[:], rhs=xT[:], start=True, stop=True)
        h1_sb = const.tile([D, b_loc], F16, tag="h1")
        nc.scalar.activation(h1_sb[:], h1_ps[:], AF.Relu, bias=b1col[:], scale=1.0)
        lg_ps = ps_a.tile([1, b_loc], F32, tag="pa")
        nc.tensor.matmul(lg_ps[:], lhsT=w2_sb[:], rhs=h1_sb[:], start=True, stop=True)
        lg_sb = const.tile([1, b_loc], F32, tag="lg")
        nc.scalar.activation(lg_sb[:], lg_ps[:], AF.Identity, bias=b2_sb[:], scale=1.0)
        nc.sync.dma_start(out=logit[:], in_=lg_sb[:])

    nc.compile()
    return nc


def make_in_maps(inputs, b_loc=B_LOC, n_cores=N_CORES):
    """Shard full inputs into per-core in_maps (data parallel over batch)."""
    idx = np.asarray(inputs["indices"]).astype(np.int32)
    # The kernel's math is invariant to the order of each sample's K gathered
    # rows (softmax axis is reduced symmetrically), so sort them: the gather's
    # HBM reads become near-sequential instead of random.
    idx = np.sort(idx, axis=1)
    # SWDGE wrapped layout: idx16[p, (g, s, cj)] = idx[32g+s, 16cj+p] + 1024 s
    # (row index into the group's [32*1024, 64] useq slice), int16, replicated
    # across the 8 16-partition blocks.
    n_grp = b_loc // 32
    idxg = idx.reshape(n_cores * n_grp, 32, K // 16, 16) + \
        (L * np.arange(32, dtype=np.int32))[None, :, None, None]
    idx16_all = np.ascontiguousarray(
        idxg.transpose(3, 0, 1, 2)).astype(np.int16)  # [16, G, 32, 8]
    idx16_all = np.broadcast_to(
        idx16_all[None], (8,) + idx16_all.shape)  # replicate to 128 partitions
    useq = np.ascontiguousarray(np.asarray(inputs["user_seq_emb"], dtype=np.float32))
    tgt = np.ascontiguousarray(np.asarray(inputs["target_emb"], dtype=np.float32)[:, 0, :])
    shared = {
        "wq": np.ascontiguousarray(np.asarray(inputs["WQ"], np.float32).reshape(H * D, D)),
        "wk": np.ascontiguousarray(np.asarray(inputs["WK"], np.float32).reshape(H * D, D)),
        "wv": np.ascontiguousarray(np.asarray(inputs["WV"], np.float32).reshape(H * D, D)),
        "bq": np.ascontiguousarray(np.asarray(inputs["bQ"], np.float32)),
        "bv": np.ascontiguousarray(np.asarray(inputs["bV"], np.float32)),
        "wo": np.ascontiguousarray(np.asarray(inputs["WO"], np.float32)),
        "bo": np.asarray(inputs["bO"], np.float32).reshape(D, 1).copy(),
        "w1": np.ascontiguousarray(np.asarray(inputs["W1"], np.float32)),
        "b1": np.asarray(inputs["b1"], np.float32).reshape(D, 1).copy(),
        "w2": np.ascontiguousarray(np.asarray(inputs["W2"], np.float32)),
        "b2": np.asarray(inputs["b2"], np.float32).reshape(1, 1).copy(),
    }
    in_maps = []
    for c in range(n_cores):
        s = slice(c * b_loc, (c + 1) * b_loc)
        m = dict(shared)
        m["useq"] = useq[s].reshape(b_loc * L, D)
        m["tgt"] = tgt[s]
        m["idx16"] = np.ascontiguousarray(
            idx16_all[:, :, c * n_grp:(c + 1) * n_grp]).reshape(128, b_loc * K // 16)
        in_maps.append(m)
    return in_maps


_NC_CACHE = {}


def kernel(**inputs):
    if B_LOC not in _NC_CACHE:
        _NC_CACHE[B_LOC] = build(B_LOC)
    nc = _NC_CACHE[B_LOC]
    in_maps = make_in_maps(inputs)
    res = run_bass_kernel_spmd(nc, in_maps, core_ids=list(range(N_CORES)))
    return np.concatenate([res.results[c]["logit"] for c in range(N_CORES)], axis=0)
